# revision 8
# baseline (speedup 1.0000x reference)
"""Trainium2 Bass kernel for nn_DecoderTF (masked spectrogram decode + overlap-add).

Computation (per batch m, channel c):
    masked[n, k] = inputs[m, n, k] * est_mask[m, c, n, k]          n in [0,512), k in [0,6000)
    frames[k, l] = sum_n masked[n, k] * W[n, l]                    l in [0,16)
    out[m, c, t] = overlap_and_add(frames, hop=8)                  t in [0,48008)

With L=16 and hop=8, overlap-add reduces to a two-term sum; viewing the output
as out2d[6001, 8]:
    out2d[k, j] = frames[k, j] + frames[k-1, j+8]
                = sum_n masked[n, k]   * W[n, j]
                + sum_n masked[n, k-1] * W[n, j+8]
which is two matmuls (W halves stationary, masked streaming, the second with the
moving operand shifted one column) accumulated into one PSUM tile.  The OLA
costs nothing: the PSUM tile [8, 512] IS a transposed chunk of out2d.

Sharding: data-parallel over M — core m handles inputs[m] / est_mask[m] (no
cross-core communication, W replicated).  Per-core HBM traffic ~37 MB, which is
the roofline for this kernel.
"""

import sys

for _p in ("/opt/trn_rl_repo",):
    if _p not in sys.path:
        sys.path.insert(0, _p)

import numpy as np

import concourse.bass as bass
import concourse.mybir as mybir
from concourse import bacc, masks
from concourse.tile import TileContext
from concourse.bass_utils import run_bass_kernel_spmd

N, L, HOP = 512, 16, 8
K = 6000
C = 2
M = 8
T_OUT = (K - 1) * HOP + L  # 48008
R = K + 1                  # rows of out2d: out2d[k, j] = out[k*8 + j]

F32 = mybir.dt.float32
# float32r streams fp32 bits through the PE's single-pass (reduced internal
# precision) path: 1 cycle/row vs 4 for full fp32.
MM_DT = mybir.dt.float32r

# k-slices (over out2d rows / masked columns), each split into <=512-wide PSUM
# chunks.  1536 = 3 chunks of 512 and a multiple of 128 (transpose sub-tiles).
KSLICES = [(0, 1536), (1536, 1536), (3072, 1536), (4608, R - 4608)]  # last: 1393
MAXW = 1537  # widest sbuf tile: 1536 + 1 halo column


def _build_nc():
    nc = bacc.Bacc()
    x = nc.declare_dram_parameter("x", [N, K], F32, isOutput=False)
    mk = nc.declare_dram_parameter("mask", [C, N, K], F32, isOutput=False)
    w = nc.declare_dram_parameter("w", [N, L], MM_DT, isOutput=False)
    out = nc.declare_dram_parameter("out", [C, T_OUT], F32, isOutput=True)

    with TileContext(nc) as tc:
        with (
            tc.tile_pool(name="wp", bufs=1) as wp,
            tc.tile_pool(name="idp", bufs=1) as idp,
            tc.tile_pool(name="xp", bufs=8) as xp,
            tc.tile_pool(name="mp", bufs=4) as mp,
            tc.tile_pool(name="mkp", bufs=8) as mkp,
            tc.tile_pool(name="frp", bufs=3) as frp,
            tc.tile_pool(name="outp", bufs=1) as outp,
            tc.tile_pool(name="pop", bufs=6, space="PSUM") as pop,
            tc.tile_pool(name="ptp", bufs=2, space="PSUM") as ptp,
        ):
            # W, layout [p, 16n + l] = W[128n + p, l]; lhsT slices are 8 cols.
            w_t = wp.tile([128, 4 * L], MM_DT)
            for n in range(4):
                nc.sync.dma_start(
                    out=w_t[:, L * n : L * (n + 1)],
                    in_=w[128 * n : 128 * (n + 1), :],
                )
            id_t = idp.tile([8, 8], F32)
            masks.make_identity(nc, id_t[:, :])

            # out_sb[p, 376c + 8S + j] = out2d[128S + p, j], S = global subtile
            out_sb = outp.tile([128, 2 * 376], F32)

            for ks_i, (o0, wks) in enumerate(KSLICES):
                # tile col j holds masked col (base + j); base = o0-1 so the
                # B-term (shift-by-one) always starts at tile col >= 0.  For
                # ks 0, tile col 0 is a virtual masked[-1] == 0 column.
                base = o0 - 1
                hi = min(o0 + wks, K)
                vt = hi - base               # total tile cols in use
                doff = 0 if o0 > 0 else 1    # col where DMA'd data starts
                dlen = hi - max(base, 0)
                x_ts = []
                for n in range(4):
                    x_t = xp.tile([128, MAXW], F32, tag="x")
                    if doff:
                        nc.gpsimd.memset(x_t[:, 0:doff], 0.0)
                    nc.sync.dma_start(
                        out=x_t[:, doff : doff + dlen],
                        in_=x[128 * n : 128 * (n + 1), max(base, 0) : hi],
                    )
                    x_ts.append(x_t)

                chunks = []
                q0 = o0
                while q0 < o0 + wks:
                    chunks.append((q0, min(512, o0 + wks - q0)))
                    q0 += 512

                for c in range(C):
                    mk_ts = []
                    for n in range(4):
                        m_t = mp.tile([128, MAXW], F32, tag="m")
                        nc.sync.dma_start(
                            out=m_t[:, doff : doff + dlen],
                            in_=mk[c, 128 * n : 128 * (n + 1), max(base, 0) : hi],
                        )
                        mk_t = mkp.tile([128, MAXW], MM_DT, tag="mk")
                        if doff:
                            nc.gpsimd.memset(m_t[:, 0:doff], 0.0)
                        nc.vector.tensor_mul(
                            mk_t[:, 0:vt], x_ts[n][:, 0:vt], m_t[:, 0:vt]
                        )
                        mk_ts.append(mk_t)

                    fr_t = frp.tile([8, 1536], F32, tag="fr")
                    for q0, wch in chunks:
                        po = pop.tile([8, 512], F32, tag="po")
                        wa = min(wch, K - q0)  # A-term covers [0, wa)
                        # B(n=0) first: it covers the full [0, wch) width
                        ops = [(0, 1), (0, 0)] + [
                            (n, half) for n in range(1, 4) for half in (1, 0)
                        ]
                        for i, (n, half) in enumerate(ops):
                            st, sp = i == 0, i == len(ops) - 1
                            if half == 0:
                                nc.tensor.matmul(
                                    po[0:8, 0:wa],
                                    w_t[:, L * n : L * n + 8],
                                    mk_ts[n][:, q0 - base : q0 - base + wa],
                                    start=st, stop=sp,
                                )
                            else:
                                # pad odd widths to even (fp32 PSUM writes are
                                # 8-byte granular); the pad column is never read
                                wb = wch + (wch & 1)
                                nc.tensor.matmul(
                                    po[0:8, 0:wb],
                                    w_t[:, L * n + 8 : L * n + 16],
                                    mk_ts[n][:, q0 - 1 - base : q0 - 1 - base + wb],
                                    start=st, stop=sp,
                                )
                        nc.scalar.copy(
                            fr_t[:, q0 - o0 : q0 - o0 + wch], po[0:8, 0:wch]
                        )

                    # transpose [8, 128] slabs -> [128, 8] into one PSUM bank
                    n_sub = (wks + 127) // 128
                    pt = ptp.tile([128, 96], F32, tag="pt")
                    for s in range(n_sub):
                        s0 = 128 * s
                        sw = min(128, wks - s0)
                        nc.tensor.transpose(
                            pt[0:sw, 8 * s : 8 * s + 8],
                            fr_t[:, s0 : s0 + sw],
                            id_t[:, :],
                        )
                    nc.scalar.copy(
                        out_sb[:, 376 * c + 96 * ks_i : 376 * c + 96 * ks_i + 8 * n_sub],
                        pt[:, 0 : 8 * n_sub],
                    )

            for c in range(C):
                # rows 0..5887 of out2d: fully dense [46, 128, 8] blocks
                v1 = out[c, 0 : 46 * 1024].rearrange("(s p j) -> p s j", p=128, j=8)
                s1 = out_sb[:, 376 * c : 376 * c + 368].rearrange(
                    "p (s j) -> p s j", j=8
                )
                nc.sync.dma_start(out=v1, in_=s1)
                # rows 5888..6000 (113 rows) from the last subtile
                v2 = out[c, 46 * 1024 : T_OUT].rearrange("(p j) -> p j", j=8)
                nc.sync.dma_start(
                    out=v2, in_=out_sb[0:113, 376 * c + 368 : 376 * c + 376]
                )
    nc.finalize()
    return nc


_NC_CACHE = None


def _get_nc():
    global _NC_CACHE
    if _NC_CACHE is None:
        _NC_CACHE = _build_nc()
    return _NC_CACHE


def run(inputs, est_mask, W, trace=False):
    """Returns (out [M, C, T_OUT] float32, exec_time_ns or None)."""
    inputs = np.ascontiguousarray(np.asarray(inputs, dtype=np.float32))
    est_mask = np.ascontiguousarray(np.asarray(est_mask, dtype=np.float32))
    W = np.ascontiguousarray(np.asarray(W, dtype=np.float32))
    assert inputs.shape == (M, N, K)
    assert est_mask.shape == (M, C, N, K)
    assert W.shape == (N, L)

    nc = _get_nc()
    in_maps = [
        {"x": inputs[m], "mask": est_mask[m], "w": W} for m in range(M)
    ]
    res = run_bass_kernel_spmd(nc, in_maps, list(range(M)), trace=trace)
    out = np.stack([res.results[m]["out"] for m in range(M)], axis=0)
    return out.astype(np.float32, copy=False), res.exec_time_ns


def kernel(inputs, est_mask, W):
    out, _ = run(inputs, est_mask, W)
    return out


# revision 13
# speedup vs baseline: 1.0196x; 1.0196x over previous
"""Trainium2 Bass kernel for nn_DecoderTF (masked spectrogram decode + overlap-add).

Computation (per batch m, channel c):
    masked[n, k] = inputs[m, n, k] * est_mask[m, c, n, k]          n in [0,512), k in [0,6000)
    frames[k, l] = sum_n masked[n, k] * W[n, l]                    l in [0,16)
    out[m, c, t] = overlap_and_add(frames, hop=8)                  t in [0,48008)

With L=16 and hop=8, overlap-add reduces to a two-term sum; viewing the output
as out2d[6001, 8]:
    out2d[k, j] = frames[k, j] + frames[k-1, j+8]
                = sum_n masked[n, k]   * W[n, j]
                + sum_n masked[n, k-1] * W[n, j+8]
which is two matmuls (W halves stationary, masked streaming, the second with the
moving operand shifted one column) accumulated into one PSUM tile.  The OLA
costs nothing: the PSUM tile [8, 512] IS a transposed chunk of out2d.

Sharding: data-parallel over M — core m handles inputs[m] / est_mask[m] (no
cross-core communication, W replicated).  Per-core HBM traffic ~37 MB, which is
the roofline for this kernel.
"""

import sys

for _p in ("/opt/trn_rl_repo",):
    if _p not in sys.path:
        sys.path.insert(0, _p)

import numpy as np

import concourse.bass as bass
import concourse.mybir as mybir
from concourse import bacc, masks
from concourse.tile import TileContext
from concourse.bass_utils import run_bass_kernel_spmd

N, L, HOP = 512, 16, 8
K = 6000
C = 2
M = 8
T_OUT = (K - 1) * HOP + L  # 48008
R = K + 1                  # rows of out2d: out2d[k, j] = out[k*8 + j]

F32 = mybir.dt.float32
# float32r streams fp32 bits through the PE's single-pass (reduced internal
# precision) path: 1 cycle/row vs 4 for full fp32.
MM_DT = mybir.dt.float32r

# k-slices (over out2d rows / masked columns), each split into <=512-wide PSUM
# chunks.  1536 = 3 chunks of 512 and a multiple of 128 (transpose sub-tiles).
KSLICES = [(0, 1536), (1536, 1536), (3072, 1536), (4608, R - 4608)]  # last: 1393
MAXW = 1537  # widest sbuf tile: 1536 + 1 halo column


def _build_nc():
    nc = bacc.Bacc()
    x = nc.declare_dram_parameter("x", [N, K], F32, isOutput=False)
    mk = nc.declare_dram_parameter("mask", [C, N, K], F32, isOutput=False)
    w = nc.declare_dram_parameter("w", [N, L], MM_DT, isOutput=False)
    out = nc.declare_dram_parameter("out", [C, T_OUT], F32, isOutput=True)

    with TileContext(nc) as tc:
        with (
            tc.tile_pool(name="wp", bufs=1) as wp,
            tc.tile_pool(name="idp", bufs=1) as idp,
            tc.tile_pool(name="xp", bufs=6) as xp,
            tc.tile_pool(name="mp", bufs=5) as mp,
            tc.tile_pool(name="mkp", bufs=9) as mkp,
            tc.tile_pool(name="frp", bufs=3) as frp,
            tc.tile_pool(name="outp", bufs=1) as outp,
            tc.tile_pool(name="pop", bufs=6, space="PSUM") as pop,
            tc.tile_pool(name="ptp", bufs=2, space="PSUM") as ptp,
        ):
            # W, layout [p, 16n + l] = W[128n + p, l]; lhsT slices are 8 cols.
            w_t = wp.tile([128, 4 * L], MM_DT)
            nc.sync.dma_start(
                out=w_t[:, :].rearrange("p (n l) -> p n l", n=4),
                in_=w.rearrange("(n p) l -> p n l", p=128),
            )
            id_t = idp.tile([8, 8], F32)
            masks.make_identity(nc, id_t[:, :])

            # out_sb[p, 376c + 8S + j] = out2d[128S + p, j], S = global subtile
            out_sb = outp.tile([128, 2 * 376], F32)

            for ks_i, (o0, wks) in enumerate(KSLICES):
                # tile col j holds masked col (base + j); base = o0-1 so the
                # B-term (shift-by-one) always starts at tile col >= 0.  For
                # ks 0, tile col 0 is a virtual masked[-1] == 0 column.
                base = o0 - 1
                hi = min(o0 + wks, K)
                vt = hi - base               # total tile cols in use
                doff = 0 if o0 > 0 else 1    # col where DMA'd data starts
                dlen = hi - max(base, 0)
                mk_ts = {}
                for n in range(4):
                    x_t = xp.tile([128, MAXW], F32, tag="x")
                    if doff:
                        nc.gpsimd.memset(x_t[:, 0:doff], 0.0)
                    nc.sync.dma_start(
                        out=x_t[:, doff : doff + dlen],
                        in_=x[128 * n : 128 * (n + 1), max(base, 0) : hi],
                    )
                    # both channels' masks in one DMA: dram [c, p, k] -> [p, c, k]
                    m_t = mp.tile([128, 2 * MAXW], F32, tag="m")
                    if doff:
                        nc.gpsimd.memset(m_t[:, 0:doff], 0.0)
                        nc.gpsimd.memset(m_t[:, MAXW : MAXW + doff], 0.0)
                    nc.sync.dma_start(
                        out=m_t[:, :].rearrange("p (c k) -> p c k", c=2)[
                            :, :, doff : doff + dlen
                        ],
                        in_=mk[:, 128 * n : 128 * (n + 1), max(base, 0) : hi].transpose(
                            [1, 0, 2]
                        ),
                    )
                    for c in range(C):
                        mk_t = mkp.tile([128, MAXW], MM_DT, tag="mk")
                        nc.vector.tensor_mul(
                            mk_t[:, 0:vt],
                            x_t[:, 0:vt],
                            m_t[:, c * MAXW : c * MAXW + vt],
                        )
                        mk_ts[c, n] = mk_t

                chunks = []
                q0 = o0
                while q0 < o0 + wks:
                    chunks.append((q0, min(512, o0 + wks - q0)))
                    q0 += 512

                for c in range(C):
                    fr_t = frp.tile([8, 1536], F32, tag="fr")
                    for q0, wch in chunks:
                        po = pop.tile([8, 512], F32, tag="po")
                        wa = min(wch, K - q0)  # A-term covers [0, wa)
                        # B(n=0) first: it covers the full [0, wch) width
                        ops = [(0, 1), (0, 0)] + [
                            (n, half) for n in range(1, 4) for half in (1, 0)
                        ]
                        for i, (n, half) in enumerate(ops):
                            st, sp = i == 0, i == len(ops) - 1
                            if half == 0:
                                nc.tensor.matmul(
                                    po[0:8, 0:wa],
                                    w_t[:, L * n : L * n + 8],
                                    mk_ts[c, n][:, q0 - base : q0 - base + wa],
                                    start=st, stop=sp,
                                )
                            else:
                                # pad odd widths to even (fp32 PSUM writes are
                                # 8-byte granular); the pad column is never read
                                wb = wch + (wch & 1)
                                nc.tensor.matmul(
                                    po[0:8, 0:wb],
                                    w_t[:, L * n + 8 : L * n + 16],
                                    mk_ts[c, n][:, q0 - 1 - base : q0 - 1 - base + wb],
                                    start=st, stop=sp,
                                )
                        nc.scalar.copy(
                            fr_t[:, q0 - o0 : q0 - o0 + wch], po[0:8, 0:wch]
                        )

                    # transpose [8, 128] slabs -> [128, 8] into one PSUM bank
                    n_sub = (wks + 127) // 128
                    pt = ptp.tile([128, 96], F32, tag="pt")
                    for s in range(n_sub):
                        s0 = 128 * s
                        sw = min(128, wks - s0)
                        nc.tensor.transpose(
                            pt[0:sw, 8 * s : 8 * s + 8],
                            fr_t[:, s0 : s0 + sw],
                            id_t[:, :],
                        )
                    ob = 376 * c + 96 * ks_i
                    nc.scalar.copy(
                        out_sb[:, ob : ob + 8 * n_sub], pt[:, 0 : 8 * n_sub]
                    )
                    # stream this slice's output rows out now; the final
                    # 113-row partial subtile goes separately
                    s_full = n_sub if wks % 128 == 0 else n_sub - 1
                    t0 = 1024 * (12 * ks_i)
                    v = out[c, t0 : t0 + 1024 * s_full].rearrange(
                        "(s p j) -> p s j", p=128, j=8
                    )
                    sv = out_sb[:, ob : ob + 8 * s_full].rearrange(
                        "p (s j) -> p s j", j=8
                    )
                    nc.sync.dma_start(out=v, in_=sv)
                    if s_full != n_sub:
                        rem = wks - 128 * s_full  # 113
                        v2 = out[c, t0 + 1024 * s_full : T_OUT].rearrange(
                            "(p j) -> p j", j=8
                        )
                        nc.sync.dma_start(
                            out=v2,
                            in_=out_sb[0:rem, ob + 8 * s_full : ob + 8 * n_sub],
                        )
    nc.finalize()
    return nc


_NC_CACHE = None


def _get_nc():
    global _NC_CACHE
    if _NC_CACHE is None:
        _NC_CACHE = _build_nc()
    return _NC_CACHE


def run(inputs, est_mask, W, trace=False):
    """Returns (out [M, C, T_OUT] float32, exec_time_ns or None)."""
    inputs = np.ascontiguousarray(np.asarray(inputs, dtype=np.float32))
    est_mask = np.ascontiguousarray(np.asarray(est_mask, dtype=np.float32))
    W = np.ascontiguousarray(np.asarray(W, dtype=np.float32))
    assert inputs.shape == (M, N, K)
    assert est_mask.shape == (M, C, N, K)
    assert W.shape == (N, L)

    nc = _get_nc()
    in_maps = [
        {"x": inputs[m], "mask": est_mask[m], "w": W} for m in range(M)
    ]
    res = run_bass_kernel_spmd(nc, in_maps, list(range(M)), trace=trace)
    out = np.stack([res.results[m]["out"] for m in range(M)], axis=0)
    return out.astype(np.float32, copy=False), res.exec_time_ns


def kernel(inputs, est_mask, W):
    out, _ = run(inputs, est_mask, W)
    return out
